# revision 15
# baseline (speedup 1.0000x reference)
"""GAT 2-layer GNN kernel for Trainium2, 8 NeuronCores.

v3 strategy (edge-sharded by destination, diagonal slot layout, fp16 tables,
dma_gather):
  - Nodes sorted by in-degree desc, dealt round-robin to 8 cores; table row
    id = core*6272 + pos (dummy row at pos 6250). NT=50176 rows.
  - Phase A: every core computes the full node table h1ext = x@[W1|W1As|W1Ad]
    fp16 (PE otherwise idle; no big AllGather). Rows are 768B-strided
    [h1(256)|a_src(8)|pad], only the first 528B written. Dummy rows hold
    h=0, a_src=-30000 => zero attention weight; slot padding points at them,
    so no masks exist anywhere.
  - Edge gather via the InstDMAGatherAnt ucode: one instruction fetches 1024
    rows (8 slot-columns x 128 partitions), out[p,j]=tab[idx[j*128+p]],
    amortizing the ~1us SWDGE fixed cost ~8x vs per-slot indirect DMA.
    Indices are int16, so gathers read from two overlapping 32768-row
    windows (lo=[0,32768), hi=[17408,50176)); each dst's edges are
    host-balanced between the windows (~10% extra slots).
  - Slot axis is in gather-column order: [lo slots block-major | hi slots
    block-major]. DVE: leaky-relu logits, exact per-dst segment max (two
    region reduces + combine), scalar-engine exp broadcast-written into a
    packed [slots*heads, ch] fp16 tile (alpha*h multiply at DVE 2x),
    in-place pairwise add-tree over slots per region (2x), combine. Evac
    normalizes, adds b1, ELU, PE-transposes x2 for layer 2.
  - Phase C: layer-2 slice (256B-strided rows [h2(32)|a_src2(1)|pad]), small
    AllGather; phase D mirrors B.
  - b1/b2 are added AFTER aggregation (sum alpha=1); their constant logit
    shifts cancel in softmax.
"""

import numpy as np

# ---------------------------------------------------------------- problem cfg
N = 50000
E = 800000
F_IN = 128
H = 8
CH = 32
F2 = H * CH  # 256
COUT = 32
NCORES = 8
P = 128
NPC = N // NCORES  # 6250
NBLK = 49
NPC1 = NBLK * P  # 6272
NT = NPC1 * NCORES  # 50176
ROW1 = F2 + H  # 264 used cols of the L1 row
STEP1 = 384  # L1 row stride in f16 elems (768B)
ROW2 = COUT + 1  # 33 used cols of the L2 row
STEP2 = 128  # L2 row stride in f16 elems (256B)
W1C = F2 + 2 * H  # 272
W2C = COUT + 2  # 34
NEG_SLOPE = 0.2
WIN = 32768
HI0 = NT - WIN  # 17408
DUM_LO = 6250
DUM_HI = 3 * NPC1 + 6250 - HI0  # 7658
NJMAX = 8  # slot columns per dma_gather (1024 rows, HW-proven)
SMAX = 40  # max slots (lo+hi) per processing group
GMAX = 4
SBA = 8
NTB = NT // P  # 392

_CACHE = {}


# ================================================================ host prep
def _prep(edge_index):
    src = np.concatenate(
        [edge_index[0].astype(np.int64), np.arange(N, dtype=np.int64)]
    )
    dst = np.concatenate(
        [edge_index[1].astype(np.int64), np.arange(N, dtype=np.int64)]
    )
    indeg = np.bincount(dst, minlength=N)
    order = np.argsort(-indeg, kind="stable")
    posn = np.empty(N, np.int64)
    posn[order] = np.arange(N)
    core_of = posn % NCORES
    pos_of = posn // NCORES
    row_of = core_of * NPC1 + pos_of

    er = row_of[dst]
    srow = row_of[src]
    cls = (srow >= HI0).astype(np.int64) + (srow >= WIN).astype(np.int64)
    eorder = np.lexsort((cls, er))
    er_s = er[eorder]
    sr_s = srow[eorder]
    key = er_s * 3 + cls[eorder]
    starts = np.searchsorted(er_s, np.arange(NT))
    c0 = np.searchsorted(key, np.arange(NT) * 3 + 1) - starts
    c01 = np.searchsorted(key, np.arange(NT) * 3 + 2) - starts
    deg_row = np.zeros(NT, np.int64)
    deg_row[row_of] = indeg
    flex = c01 - c0
    x = np.clip((deg_row + 1) // 2 - c0, 0, flex)
    lo_n = c0 + x
    hi_n = deg_row - lo_n
    ET = src.shape[0]

    lo_rank = np.zeros(NBLK * P * NCORES, np.int64)
    hi_rank = np.zeros(NBLK * P * NCORES, np.int64)
    lo_rank[:N] = lo_n[row_of[order]]
    hi_rank[:N] = hi_n[row_of[order]]
    S_lo = np.maximum(lo_rank.reshape(NBLK, P * NCORES).max(1), 1)
    S_hi = np.maximum(hi_rank.reshape(NBLK, P * NCORES).max(1), 1)

    groups = []  # (b0, G, SL, SH)
    b = 0
    while b < NBLK:
        SL, SH = int(S_lo[b]), int(S_hi[b])
        G = 1
        while b + G < NBLK and G < GMAX:
            nl = max(SL, int(S_lo[b + G]))
            nh2 = max(SH, int(S_hi[b + G]))
            if (G + 1) * (nl + nh2) > SMAX:
                break
            SL, SH = nl, nh2
            G += 1
        groups.append((b, G, SL, SH))
        b += G

    sched = []  # per group: list of (dest_col, nj, idx_off, win)
    iw_off = 0
    for b0, G, SL, SH in groups:
        gl = []
        for reg_base, reg_cols, win in ((0, G * SL, 0), (G * SL, G * SH, 1)):
            c = 0
            while c < reg_cols:
                nj = min(NJMAX, reg_cols - c)
                gl.append((reg_base + c, nj, iw_off, win))
                iw_off += 8 * nj
                c += nj
        sched.append(tuple(gl))
    TOTW = iw_off

    d_ar = np.arange(P, dtype=np.int64)
    idx_cores = []
    for c in range(NCORES):
        idxw = np.zeros((P, TOTW), np.int16)
        for gi, (b0, G, SL, SH) in enumerate(groups):
            S = SL + SH
            cols = np.zeros((P, G * S), np.int64)
            for j in range(G):
                bb = b0 + j
                pos = bb * P + d_ar
                rows = c * NPC1 + pos
                valid = pos < NPC
                rs = np.where(valid, rows, 0)
                ln = np.where(valid, lo_n[rs], 0)
                hn = np.where(valid, hi_n[rs], 0)
                st = starts[rs]
                gs = np.arange(SL, dtype=np.int64)
                eidx = np.minimum(st[:, None] + gs, ET - 1)
                ok = valid[:, None] & (gs[None, :] < ln[:, None])
                cols[:, j * SL : (j + 1) * SL] = np.where(
                    ok, sr_s[eidx], DUM_LO
                )
                gs2 = np.arange(SH, dtype=np.int64)
                eidx2 = np.minimum(st[:, None] + ln[:, None] + gs2, ET - 1)
                ok2 = valid[:, None] & (gs2[None, :] < hn[:, None])
                cols[:, G * SL + j * SH : G * SL + (j + 1) * SH] = np.where(
                    ok2, sr_s[eidx2] - HI0, DUM_HI
                )
            for dest_col, nj, io, win in sched[gi]:
                sub = cols[:, dest_col : dest_col + nj]
                flat = sub.T.reshape(-1)  # k = j*128 + p
                wr = flat.reshape(8 * nj, 16).T  # [16, 8nj]
                for g in range(8):
                    idxw[16 * g : 16 * (g + 1), io : io + 8 * nj] = wr
        idx_cores.append(np.ascontiguousarray(idxw))

    return dict(
        groups=groups,
        sched=tuple(sched),
        totw=TOTW,
        idx=idx_cores,
        order=order,
    )


# ================================================================ device prog
def _build_program(groups, sched, totw):
    from concourse import bacc, mybir, tile

    f32 = mybir.dt.float32
    f16 = mybir.dt.float16
    i16 = mybir.dt.int16
    AF = mybir.ActivationFunctionType
    OP = mybir.AluOpType
    AX = mybir.AxisListType

    nc = bacc.Bacc(
        "TRN2", target_bir_lowering=False, debug=False, num_devices=NCORES
    )

    # -------- I/O
    xT_d = nc.dram_tensor("xT16", [F_IN, NT], f16, kind="ExternalInput")
    xo_d = nc.dram_tensor("xTown16", [F_IN, NPC1], f16, kind="ExternalInput")
    idx_d = nc.dram_tensor("idxw", [P, totw], i16, kind="ExternalInput")
    w1e_d = nc.dram_tensor("w1e16", [F_IN, W1C], f16, kind="ExternalInput")
    w2e_d = nc.dram_tensor("w2e16", [F2, W2C], f16, kind="ExternalInput")
    b1_d = nc.dram_tensor("b1rep", [P, F2], f32, kind="ExternalInput")
    b2_d = nc.dram_tensor("b2rep", [P, COUT], f32, kind="ExternalInput")
    id_d = nc.dram_tensor("ident16", [P, P], f16, kind="ExternalInput")
    out_d = nc.dram_tensor("out", [NPC1, COUT], f32, kind="ExternalOutput")

    h1f_d = nc.dram_tensor("h1full", [NT, STEP1], f16)
    h2s_d = nc.dram_tensor("h2slice", [NPC1, STEP2], f16)
    h2f_d = nc.dram_tensor("h2full", [NT, STEP2], f16, addr_space="Shared")

    cgroups = [[i for i in range(NCORES)]]

    with tile.TileContext(nc) as tc:
        from contextlib import ExitStack

        ctx = ExitStack()
        cpool = ctx.enter_context(tc.tile_pool(name="consts", bufs=1))
        rpool = ctx.enter_context(tc.tile_pool(name="resident", bufs=1))
        apool = ctx.enter_context(tc.tile_pool(name="pha", bufs=2))
        gpool = ctx.enter_context(tc.tile_pool(name="gather", bufs=2))
        wpool = ctx.enter_context(tc.tile_pool(name="work", bufs=2))
        epool = ctx.enter_context(tc.tile_pool(name="evac", bufs=2))
        pspool = ctx.enter_context(tc.tile_pool(name="psum", bufs=3, space="PSUM"))
        ps2pool = ctx.enter_context(tc.tile_pool(name="psum2", bufs=2, space="PSUM"))

        # constants
        w1e_t = cpool.tile([F_IN, W1C], f16)
        nc.sync.dma_start(out=w1e_t[:], in_=w1e_d[:])
        w2a_t = cpool.tile([P, W2C], f16)
        nc.sync.dma_start(out=w2a_t[:], in_=w2e_d[0:P, :])
        w2b_t = cpool.tile([P, W2C], f16)
        nc.sync.dma_start(out=w2b_t[:], in_=w2e_d[P : 2 * P, :])
        b1_t = cpool.tile([P, F2], f32)
        nc.sync.dma_start(out=b1_t[:], in_=b1_d[:])
        b2_t = cpool.tile([P, COUT], f32)
        nc.sync.dma_start(out=b2_t[:], in_=b2_d[:])
        id_t = cpool.tile([P, P], f16)
        nc.sync.dma_start(out=id_t[:], in_=id_d[:])

        # resident state
        adst1_t = rpool.tile([P, NBLK, H], f16)
        adst2_t = rpool.tile([P, NBLK, 1], f16)
        x2T0 = rpool.tile([P, NPC1], f16)
        x2T1 = rpool.tile([P, NPC1], f16)

        # ---------------- phase A: full h1ext table
        nsb = NTB // SBA
        for sb in range(nsb):
            xin = apool.tile([F_IN, SBA * P], f16, tag="xin")
            nc.sync.dma_start(
                out=xin[:], in_=xT_d[:, sb * SBA * P : (sb + 1) * SBA * P]
            )
            h1st = apool.tile([P, SBA, ROW1], f16, tag="h1st")
            for j in range(SBA):
                blk = sb * SBA + j
                ps = pspool.tile([P, W1C], f32, tag="psA")
                nc.tensor.matmul(
                    ps[:],
                    lhsT=xin[:, j * P : (j + 1) * P],
                    rhs=w1e_t[:],
                    start=True,
                    stop=True,
                )
                if blk % 2 == 0:
                    nc.scalar.copy(out=h1st[:, j, :], in_=ps[:, 0:ROW1])
                else:
                    nc.vector.tensor_copy(out=h1st[:, j, :], in_=ps[:, 0:ROW1])
            nc.sync.dma_start(
                out=h1f_d[sb * SBA * P : (sb + 1) * SBA * P, 0:ROW1].rearrange(
                    "(j p) c -> p j c", p=P
                ),
                in_=h1st[:],
            )

        negt = cpool.tile([NCORES, H], f16)
        nc.vector.memset(negt[:], -30000.0)
        nc.sync.dma_start(
            out=h1f_d[:].rearrange("(c q) r -> c q r", q=NPC1)[
                :, 6250, F2 : F2 + H
            ],
            in_=negt[:],
        )

        for b in range(NBLK):
            xo = apool.tile([F_IN, P], f16, tag="xo")
            nc.sync.dma_start(out=xo[:], in_=xo_d[:, b * P : (b + 1) * P])
            psd = pspool.tile([P, W1C], f32, tag="psA")
            nc.tensor.matmul(
                psd[:, 0 : 2 * H],
                lhsT=xo[:],
                rhs=w1e_t[:, F2 : F2 + 2 * H],
                start=True,
                stop=True,
            )
            nc.vector.tensor_copy(out=adst1_t[:, b, :], in_=psd[:, H : 2 * H])

        # ---------------- shared aggregation (phases B / D)
        def agg_layer(h_d, nh, ch, step, adst_t, evac_fn):
            hc = nh * ch
            for gi, (b0, G, SL, SH) in enumerate(groups):
                S = SL + SH
                GS = G * S
                GL = G * SL
                gt = gpool.tile([P, GS, step], f16, tag="gt")
                io0 = sched[gi][0][2]
                iow = sched[gi][-1][2] + 8 * sched[gi][-1][1] - io0
                idxt = apool.tile([P, 8 * SMAX], i16, tag="ixw")
                nc.sync.dma_start(
                    out=idxt[:, 0:iow], in_=idx_d[:, io0 : io0 + iow]
                )
                for dest_col, nj, io, win in sched[gi]:
                    w0 = 0 if win == 0 else HI0
                    nc.gpsimd.dma_gather(
                        out_ap=gt[:, dest_col : dest_col + nj, :],
                        in_ap=h_d[w0 : w0 + WIN, :],
                        idxs_ap=idxt[:, io - io0 : io - io0 + 8 * nj],
                        num_idxs=P * nj,
                        num_idxs_reg=P * nj,
                        elem_size=step,
                    )
                # logits (slot axis u in gt column order)
                lg = wpool.tile([P, GS, nh], f16, tag="lg")
                for base, cnt, SS in ((0, GL, SL), (GL, G * SH, SH)):
                    nc.vector.tensor_tensor(
                        out=lg[:, base : base + cnt, :].rearrange(
                            "p (g s) a -> p g s a", g=G
                        ),
                        in0=gt[:, base : base + cnt, hc : hc + nh].rearrange(
                            "p (g s) a -> p g s a", g=G
                        ),
                        in1=adst_t[:, b0 : b0 + G, :]
                        .unsqueeze(2)
                        .broadcast_to([P, G, SS, nh]),
                        op=OP.add,
                    )
                lk = wpool.tile([P, GS, nh], f16, tag="lk")
                nc.vector.scalar_tensor_tensor(
                    out=lk[:],
                    in0=lg[:],
                    scalar=NEG_SLOPE,
                    in1=lg[:],
                    op0=OP.mult,
                    op1=OP.max,
                )
                # per-dst segment max: two region reduces + combine
                mxl = wpool.tile([P, G, nh], f16, tag="mxl")
                nc.vector.tensor_reduce(
                    out=mxl[:],
                    in_=lk[:, 0:GL, :]
                    .rearrange("p (g s) a -> p g s a", g=G)
                    .transpose([0, 1, 3, 2]),
                    axis=AX.X,
                    op=OP.max,
                )
                mxh = wpool.tile([P, G, nh], f16, tag="mxh")
                nc.vector.tensor_reduce(
                    out=mxh[:],
                    in_=lk[:, GL:GS, :]
                    .rearrange("p (g s) a -> p g s a", g=G)
                    .transpose([0, 1, 3, 2]),
                    axis=AX.X,
                    op=OP.max,
                )
                nc.vector.tensor_tensor(
                    out=mxl[:], in0=mxl[:], in1=mxh[:], op=OP.max
                )
                for base, cnt, SS in ((0, GL, SL), (GL, G * SH, SH)):
                    nc.vector.tensor_tensor(
                        out=lg[:, base : base + cnt, :].rearrange(
                            "p (g s) a -> p g s a", g=G
                        ),
                        in0=lk[:, base : base + cnt, :].rearrange(
                            "p (g s) a -> p g s a", g=G
                        ),
                        in1=mxl[:].unsqueeze(2).broadcast_to([P, G, SS, nh]),
                        op=OP.subtract,
                    )
                se = wpool.tile([P, GS, nh], f16, tag="se")
                nc.scalar.activation(out=se[:], in_=lg[:], func=AF.Exp)
                ssl = epool.tile([P, G, nh], f32, tag="ssl")
                nc.vector.tensor_reduce(
                    out=ssl[:],
                    in_=se[:, 0:GL, :]
                    .rearrange("p (g s) a -> p g s a", g=G)
                    .transpose([0, 1, 3, 2]),
                    axis=AX.X,
                    op=OP.add,
                )
                ssh = epool.tile([P, G, nh], f32, tag="ssh")
                nc.vector.tensor_reduce(
                    out=ssh[:],
                    in_=se[:, GL:GS, :]
                    .rearrange("p (g s) a -> p g s a", g=G)
                    .transpose([0, 1, 3, 2]),
                    axis=AX.X,
                    op=OP.add,
                )
                nc.vector.tensor_tensor(
                    out=ssl[:], in0=ssl[:], in1=ssh[:], op=OP.add
                )
                # exp broadcast-written packed across channels
                wt = wpool.tile([P, GS * nh, ch], f16, tag="wt")
                nc.scalar.activation(
                    out=wt[:],
                    in_=lg[:]
                    .rearrange("p u a -> p (u a)")
                    .unsqueeze(2)
                    .broadcast_to([P, GS * nh, ch]),
                    func=AF.Exp,
                )
                # alpha * h, in place (2x fp16, one op; u order matches gt)
                wt4 = wt[:].rearrange("p (u a) c -> p u a c", a=nh)
                nc.vector.tensor_tensor(
                    out=wt4,
                    in0=wt4,
                    in1=gt[:, :, 0:hc].rearrange("p u (a c) -> p u a c", a=nh),
                    op=OP.mult,
                )
                # pairwise add-tree per region, then combine
                acc = epool.tile([P, G, hc], f16, tag="acc")
                wtu = wt[:].rearrange("p (u a) c -> p u (a c)", a=nh)

                def tree(base, SS):
                    tv = wtu[:, base : base + G * SS, :].rearrange(
                        "p (g s) c -> p g s c", g=G
                    )
                    cur = SS
                    while cur > 1:
                        if cur % 2:
                            nc.vector.tensor_tensor(
                                out=tv[:, :, 0:1, :],
                                in0=tv[:, :, 0:1, :],
                                in1=tv[:, :, cur - 1 : cur, :],
                                op=OP.add,
                            )
                            cur -= 1
                        hl = cur // 2
                        nc.vector.tensor_tensor(
                            out=tv[:, :, 0:hl, :],
                            in0=tv[:, :, 0:hl, :],
                            in1=tv[:, :, hl:cur, :],
                            op=OP.add,
                        )
                        cur = hl
                    return tv[:, :, 0, :]

                alo = tree(0, SL)
                ahi = tree(GL, SH)
                nc.vector.tensor_tensor(
                    out=acc[:], in0=alo, in1=ahi, op=OP.add
                )
                evac_fn(b0, G, acc[:], ssl[:])

        # ---- layer-1 evac
        def evac1(b0, G, acc, ssum):
            rs = epool.tile([P, G, H], f32, tag="rs1")
            nc.vector.reciprocal(rs[:], ssum)
            x1 = epool.tile([P, G, F2], f32, tag="x1")
            nc.vector.tensor_tensor(
                out=x1[:].rearrange("p g (a c) -> p g a c", c=CH),
                in0=acc.rearrange("p g (a c) -> p g a c", c=CH),
                in1=rs[:].unsqueeze(3).broadcast_to([P, G, H, CH]),
                op=OP.mult,
            )
            nc.vector.tensor_tensor(
                out=x1[:],
                in0=x1[:],
                in1=b1_t[:].unsqueeze(1).broadcast_to([P, G, F2]),
                op=OP.add,
            )
            tm = epool.tile([P, G, F2], f32, tag="tm")
            nc.vector.tensor_scalar_min(tm[:], x1[:], 0.0)
            nc.scalar.activation(out=tm[:], in_=tm[:], func=AF.Exp)
            x2 = epool.tile([P, G, F2], f16, tag="x2")
            nc.vector.tensor_scalar(x2[:], x1[:], 0.0, -1.0, OP.max, OP.add)
            nc.vector.tensor_tensor(out=x2[:], in0=x2[:], in1=tm[:], op=OP.add)
            for j in range(G):
                bb = b0 + j
                for half, x2T in ((0, x2T0), (1, x2T1)):
                    pst = ps2pool.tile([P, P], f16, tag="psT")
                    nc.tensor.transpose(
                        pst[:], x2[:, j, half * P : (half + 1) * P], id_t[:]
                    )
                    if half == 0:
                        nc.scalar.copy(
                            out=x2T[:, bb * P : (bb + 1) * P], in_=pst[:]
                        )
                    else:
                        nc.vector.tensor_copy(
                            out=x2T[:, bb * P : (bb + 1) * P], in_=pst[:]
                        )

        agg_layer(h1f_d, H, CH, STEP1, adst1_t, evac1)

        # ---------------- phase C: layer-2 slice + AllGather
        for b in range(NBLK):
            ps2 = pspool.tile([P, W2C], f32, tag="psA")
            nc.tensor.matmul(
                ps2[:, 0:W2C],
                lhsT=x2T0[:, b * P : (b + 1) * P],
                rhs=w2a_t[:],
                start=True,
                stop=False,
            )
            nc.tensor.matmul(
                ps2[:, 0:W2C],
                lhsT=x2T1[:, b * P : (b + 1) * P],
                rhs=w2b_t[:],
                start=False,
                stop=True,
            )
            h2t = apool.tile([P, ROW2], f16, tag="h2t")
            nc.scalar.copy(out=h2t[:], in_=ps2[:, 0:ROW2])
            nc.vector.tensor_copy(
                out=adst2_t[:, b, :], in_=ps2[:, ROW2 : ROW2 + 1]
            )
            nc.sync.dma_start(
                out=h2s_d[b * P : (b + 1) * P, 0:ROW2], in_=h2t[:]
            )
        neg2 = cpool.tile([1, ROW2], f16)
        nc.vector.memset(neg2[:, 0:COUT], 0.0)
        nc.vector.memset(neg2[:, COUT:ROW2], -30000.0)
        nc.sync.dma_start(out=h2s_d[6250 : 6251, 0:ROW2], in_=neg2[:])

        nc.gpsimd.collective_compute(
            "AllGather",
            mybir.AluOpType.bypass,
            replica_groups=cgroups,
            ins=[h2s_d[:]],
            outs=[h2f_d[:]],
        )

        # ---- layer-2 evac
        def evac2(b0, G, acc, ssum):
            rs = epool.tile([P, G, 1], f32, tag="rs2")
            nc.vector.reciprocal(rs[:], ssum)
            o1 = epool.tile([P, G, COUT], f32, tag="o1")
            nc.vector.tensor_tensor(
                out=o1[:],
                in0=acc,
                in1=rs[:].broadcast_to([P, G, COUT]),
                op=OP.mult,
            )
            nc.vector.tensor_tensor(
                out=o1[:],
                in0=o1[:],
                in1=b2_t[:].unsqueeze(1).broadcast_to([P, G, COUT]),
                op=OP.add,
            )
            tm = epool.tile([P, G, COUT], f32, tag="tm2")
            nc.vector.tensor_scalar_min(tm[:], o1[:], 0.0)
            nc.scalar.activation(out=tm[:], in_=tm[:], func=AF.Exp)
            o2 = epool.tile([P, G, COUT], f32, tag="o2")
            nc.vector.tensor_scalar(o2[:], o1[:], 0.0, -1.0, OP.max, OP.add)
            nc.vector.tensor_tensor(out=o2[:], in0=o2[:], in1=tm[:], op=OP.add)
            nc.sync.dma_start(
                out=out_d[b0 * P : (b0 + G) * P, :].rearrange(
                    "(g p) c -> p g c", p=P
                ),
                in_=o2[:],
            )

        agg_layer(h2f_d, 1, COUT, STEP2, adst2_t, evac2)

        ctx.close()

    nc.compile()
    return nc


# ================================================================ entry point
def kernel(x, edge_index, W1, att_src1, att_dst1, b1, W2, att_src2, att_dst2, b2):
    global LAST_EXEC_TIME_NS
    x = np.asarray(x, np.float32)
    edge_index = np.asarray(edge_index)
    W1 = np.asarray(W1, np.float32)
    W2 = np.asarray(W2, np.float32)

    pr = _prep(edge_index)
    groups = pr["groups"]
    sched = pr["sched"]
    totw = pr["totw"]
    order = pr["order"]

    key = (totw, tuple(groups), sched)
    if key not in _CACHE:
        _CACHE.clear()
        _CACHE[key] = _build_program(groups, sched, totw)
    nc = _CACHE[key]

    A1s = np.zeros((F2, H), np.float32)
    A1d = np.zeros((F2, H), np.float32)
    for h in range(H):
        A1s[h * CH : (h + 1) * CH, h] = np.asarray(att_src1, np.float32)[h]
        A1d[h * CH : (h + 1) * CH, h] = np.asarray(att_dst1, np.float32)[h]
    w1e = np.concatenate([W1, W1 @ A1s, W1 @ A1d], axis=1).astype(np.float16)
    w2e = np.concatenate(
        [
            W2,
            W2 @ np.asarray(att_src2, np.float32).reshape(COUT, 1),
            W2 @ np.asarray(att_dst2, np.float32).reshape(COUT, 1),
        ],
        axis=1,
    ).astype(np.float16)
    b1rep = np.broadcast_to(np.asarray(b1, np.float32), (P, F2)).copy()
    b2rep = np.broadcast_to(np.asarray(b2, np.float32), (P, COUT)).copy()
    ident = np.eye(P, dtype=np.float16)

    xperm = np.zeros((NT, F_IN), np.float32)
    for c in range(NCORES):
        nodes = order[np.arange(NPC) * NCORES + c]
        xperm[c * NPC1 : c * NPC1 + NPC] = x[nodes]
    xT16 = np.ascontiguousarray(xperm.T.astype(np.float16))

    in_maps = []
    for c in range(NCORES):
        in_maps.append(
            dict(
                xT16=xT16,
                xTown16=np.ascontiguousarray(
                    xT16[:, c * NPC1 : (c + 1) * NPC1]
                ),
                idxw=pr["idx"][c],
                w1e16=w1e,
                w2e16=w2e,
                b1rep=b1rep,
                b2rep=b2rep,
                ident16=ident,
            )
        )

    from concourse.bass_utils import run_bass_kernel_spmd

    res = run_bass_kernel_spmd(
        nc, in_maps, core_ids=list(range(NCORES)), trace=False
    )
    LAST_EXEC_TIME_NS = res.exec_time_ns

    out = np.empty((N, COUT), np.float32)
    for c in range(NCORES):
        nodes = order[np.arange(NPC) * NCORES + c]
        out[nodes] = res.results[c]["out"][:NPC]
    return out


LAST_EXEC_TIME_NS = None
